# revision 7
# baseline (speedup 1.0000x reference)
"""Trainium2 Bass kernel for DifferentiableRankIntegration (soft top-k ranking).

Math (per row r of the [B,B] similarity matrix, independently per row):
    sig[a,b]    = sigmoid((s[r,a] - s[r,b]) / tau)
    rank_pos[b] = 1 + sum_a sig[a,b] * nf[r,a]
    rank_neg[b] = 1 + sum_a sig[a,b] * pf[r,a]
    rank[b]     = rank_pos[b]*pf[r,b] + rank_neg[b]*nf[r,b]
    S_hat       = (K+1) * (w_v/(K+rank_v) + w_l/(K+rank_l))

Sharding: data-parallel over rows; each of the 8 cores takes B/8 = 128 rows
and needs only its own rows of every input (the pairwise interaction is
within-row), so there is no replication and no communication.

Per-core compute layout ("fixed-a" scheme): the 128-row block s_sc = s/tau
lives in SBUF as [128 rows, 1024 cols].  For each a in [0,1024):
  - one ScalarE ACTIVATE computes sig_a[r,b] = sigmoid(-s_sc[r,b] + s_sc[r,a])
    for all 128 rows x 1024 b in one instruction, using the per-partition
    bias operand (bias = column a of s_sc) -- no transpose, no broadcast.
  - TensorE accumulates nf[r,a]*sig_a[r,b] and pf[r,a]*sig_a[r,b] into PSUM
    via matmuls whose stationary operand is diag(nf[:,a]) / diag(pf[:,a])
    (built per-a on VectorE as identity * per-partition-scalar).
"""

import os
import numpy as np

TAU = 0.1
K = 60.0
B = 1024
N_CORES = 8
ROWS = B // N_CORES  # 128

_CACHE = {}


def _build(n_a):
    from contextlib import ExitStack

    import concourse.bass as bass
    import concourse.bacc as bacc
    import concourse.mybir as mybir
    from concourse import tile

    f32 = mybir.dt.float32
    bf16 = mybir.dt.bfloat16
    AF = mybir.ActivationFunctionType
    OP = mybir.AluOpType

    nc = bacc.Bacc("TRN2", num_devices=N_CORES)

    dins = {
        n: nc.dram_tensor(n, [ROWS, B], f32, kind="ExternalInput")
        for n in ["sv", "sl", "pf", "nf", "wv", "wl"]
    }
    ident_d = nc.dram_tensor("ident", [128, 128], bf16, kind="ExternalInput")
    out_d = nc.dram_tensor("out", [ROWS, B], f32, kind="ExternalOutput")

    with tile.TileContext(nc) as tc:
        with ExitStack() as ctx:
            cpool = ctx.enter_context(tc.tile_pool(name="const", bufs=1))
            sigp = ctx.enter_context(tc.tile_pool(name="sig", bufs=6))
            diagp = ctx.enter_context(tc.tile_pool(name="diag", bufs=6))
            psump = ctx.enter_context(tc.tile_pool(name="acc", bufs=1, space="PSUM"))
            epi = ctx.enter_context(tc.tile_pool(name="epi", bufs=1))

            sb = {}
            for n in ["sv", "sl", "pf", "nf", "wv", "wl"]:
                t = cpool.tile([ROWS, B], f32, tag=n, name=n)
                nc.gpsimd.dma_start(t[:], dins[n][:, :])
                sb[n] = t
            ident = cpool.tile([128, 128], bf16, tag="ident")
            nc.gpsimd.dma_start(ident[:], ident_d[:, :])

            accs = {}
            for n in ["pv", "nv", "pl", "nl"]:
                accs[n] = psump.tile([ROWS, B], f32, tag=n, name=n)

            for a in range(n_a):
                dN = diagp.tile([128, 128], bf16, tag="diag")
                nc.vector.tensor_scalar_mul(dN[:], ident[:], sb["nf"][:, a : a + 1])
                dP = diagp.tile([128, 128], bf16, tag="diag")
                nc.vector.tensor_scalar_mul(dP[:], ident[:], sb["pf"][:, a : a + 1])
                for sn, ap_, an_ in (("sv", "pv", "nv"), ("sl", "pl", "nl")):
                    st = sb[sn]
                    sig = sigp.tile([ROWS, B], bf16, tag="sig", name="sig")
                    nc.scalar.activation(
                        sig[:], st[:], AF.Sigmoid, bias=st[:, a : a + 1], scale=-1.0
                    )
                    for h in (0, 512):
                        nc.tensor.matmul(
                            accs[ap_][:, h : h + 512],
                            dN[:],
                            sig[:, h : h + 512],
                            start=(a == 0),
                            stop=(a == n_a - 1),
                        )
                        nc.tensor.matmul(
                            accs[an_][:, h : h + 512],
                            dP[:],
                            sig[:, h : h + 512],
                            start=(a == 0),
                            stop=(a == n_a - 1),
                        )

            # Epilogue: S_hat = (K+1) * (wv/(K+rank_v) + wl/(K+rank_l))
            terms = []
            for ap_, an_, wn in (("pv", "nv", "wv"), ("pl", "nl", "wl")):
                tp = epi.tile([ROWS, B], f32, tag="tp" + wn)
                nc.vector.scalar_tensor_tensor(
                    tp[:], accs[ap_][:], 1.0, sb["pf"][:], OP.add, OP.mult
                )
                tn = epi.tile([ROWS, B], f32, tag="tn" + wn)
                nc.vector.scalar_tensor_tensor(
                    tn[:], accs[an_][:], 1.0, sb["nf"][:], OP.add, OP.mult
                )
                den = epi.tile([ROWS, B], f32, tag="den" + wn)
                nc.vector.scalar_tensor_tensor(
                    den[:], tp[:], K, tn[:], OP.add, OP.add
                )
                inv = epi.tile([ROWS, B], f32, tag="inv" + wn)
                nc.vector.reciprocal(inv[:], den[:])
                term = epi.tile([ROWS, B], f32, tag="term" + wn)
                nc.vector.tensor_mul(term[:], inv[:], sb[wn][:])
                terms.append(term)
            ssum = epi.tile([ROWS, B], f32, tag="ssum")
            nc.vector.tensor_add(ssum[:], terms[0][:], terms[1][:])
            res = epi.tile([ROWS, B], f32, tag="res")
            nc.vector.tensor_scalar_mul(res[:], ssum[:], K + 1.0)
            nc.sync.dma_start(out_d[:, :], res[:])

    nc.compile()
    return nc


def _get_nc(n_a):
    if n_a not in _CACHE:
        _CACHE[n_a] = _build(n_a)
    return _CACHE[n_a]


def run(s_v, s_l, pos_mask, neg_mask, w_v, w_l, n_a=B, trace=False):
    import ml_dtypes
    from concourse.bass_utils import run_bass_kernel_spmd

    sv = (np.asarray(s_v, dtype=np.float32) / TAU).astype(np.float32)
    sl = (np.asarray(s_l, dtype=np.float32) / TAU).astype(np.float32)
    pf = np.asarray(pos_mask).astype(np.float32)
    nf = np.asarray(neg_mask).astype(np.float32)
    wv = np.asarray(w_v, dtype=np.float32)
    wl = np.asarray(w_l, dtype=np.float32)
    ident = np.eye(128, dtype=ml_dtypes.bfloat16)

    in_maps = []
    for i in range(N_CORES):
        r = slice(i * ROWS, (i + 1) * ROWS)
        in_maps.append(
            {
                "sv": sv[r],
                "sl": sl[r],
                "pf": pf[r],
                "nf": nf[r],
                "wv": wv[r],
                "wl": wl[r],
                "ident": ident,
            }
        )

    nc = _get_nc(n_a)
    br = run_bass_kernel_spmd(nc, in_maps, list(range(N_CORES)), trace=trace)
    out = np.concatenate([br.results[i]["out"] for i in range(N_CORES)], axis=0)
    return out, br


def kernel(s_v, s_l, pos_mask, neg_mask, w_v, w_l):
    out, _ = run(s_v, s_l, pos_mask, neg_mask, w_v, w_l)
    return out


def _prep_in_maps(s_v, s_l, pos_mask, neg_mask, w_v, w_l):
    import ml_dtypes

    sv = (np.asarray(s_v, dtype=np.float32) / TAU).astype(np.float32)
    sl = (np.asarray(s_l, dtype=np.float32) / TAU).astype(np.float32)
    pf = np.asarray(pos_mask).astype(np.float32)
    nf = np.asarray(neg_mask).astype(np.float32)
    wv = np.asarray(w_v, dtype=np.float32)
    wl = np.asarray(w_l, dtype=np.float32)
    ident = np.eye(128, dtype=ml_dtypes.bfloat16)
    in_maps = []
    for i in range(N_CORES):
        r = slice(i * ROWS, (i + 1) * ROWS)
        in_maps.append(
            {"sv": sv[r], "sl": sl[r], "pf": pf[r], "nf": nf[r],
             "wv": wv[r], "wl": wl[r], "ident": ident}
        )
    return in_maps


def make_runner(nc, in_maps):
    """Persistent jitted runner for repeat-timing (mirrors run_bass_via_pjrt)."""
    import jax
    import concourse.mybir as mybir
    from jax.experimental.shard_map import shard_map
    from jax.sharding import Mesh, NamedSharding, PartitionSpec
    from concourse import bass2jax

    bass2jax.install_neuronx_cc_hook()
    n_cores = len(in_maps)
    partition_name = nc.partition_id_tensor.name if nc.partition_id_tensor else None

    in_names, out_names, out_avals, zero_outs = [], [], [], []
    for alloc in nc.m.functions[0].allocations:
        if not isinstance(alloc, mybir.MemoryLocationSet):
            continue
        name = alloc.memorylocations[0].name
        if alloc.kind == "ExternalInput":
            if name != partition_name:
                in_names.append(name)
        elif alloc.kind == "ExternalOutput":
            shape = tuple(alloc.tensor_shape)
            dtype = mybir.dt.np(alloc.dtype)
            out_names.append(name)
            out_avals.append(jax.core.ShapedArray(shape, dtype))
            zero_outs.append(np.zeros(shape, dtype))
    n_params = len(in_names)
    all_in_names = list(in_names) + list(out_names)

    def _body(*args):
        operands = list(args)
        if partition_name is not None:
            operands.append(bass2jax.partition_id_tensor())
        bind_in_names = all_in_names + ([partition_name] if partition_name else [])
        outs = bass2jax._bass_exec_p.bind(
            *operands,
            out_avals=tuple(out_avals),
            in_names=tuple(bind_in_names),
            out_names=tuple(out_names),
            lowering_input_output_aliases=(),
            sim_require_finite=True,
            sim_require_nnan=True,
            nc=nc,
        )
        return tuple(outs)

    devices = jax.devices()[:n_cores]
    mesh = Mesh(np.asarray(devices), ("core",))
    spec = PartitionSpec("core")
    sharded = jax.jit(
        shard_map(
            _body,
            mesh=mesh,
            in_specs=(spec,) * (n_params + len(out_names)),
            out_specs=(spec,) * len(out_names),
            check_rep=False,
        ),
        keep_unused=True,
    )
    sharding = NamedSharding(mesh, spec)
    concat_in = [
        jax.device_put(
            np.concatenate([np.asarray(in_maps[c][n]) for c in range(n_cores)], axis=0),
            sharding,
        )
        for n in in_names
    ]
    concat_zeros = [
        jax.device_put(np.zeros((n_cores * z.shape[0], *z.shape[1:]), z.dtype), sharding)
        for z in zero_outs
    ]

    def exec_once():
        outs = sharded(*concat_in, *concat_zeros)
        jax.block_until_ready(outs)
        return outs

    def fetch(outs):
        res = []
        for c in range(n_cores):
            d = {
                name: np.asarray(outs[i]).reshape(n_cores, *out_avals[i].shape)[c]
                for i, name in enumerate(out_names)
            }
            res.append(d)
        return res

    return exec_once, fetch


# revision 14
# speedup vs baseline: 1.1950x; 1.1950x over previous
"""Trainium2 Bass kernel for DifferentiableRankIntegration (soft top-k ranking).

Math (per row r of the [B,B] similarity matrix, independently per row):
    sig[a,b]    = sigmoid((s[r,a] - s[r,b]) / tau)
    rank_pos[b] = 1 + sum_a sig[a,b] * nf[r,a]
    rank_neg[b] = 1 + sum_a sig[a,b] * pf[r,a]
    rank[b]     = rank_pos[b]*pf[r,b] + rank_neg[b]*nf[r,b]
    S_hat       = (K+1) * (w_v/(K+rank_v) + w_l/(K+rank_l))

Sharding: data-parallel over rows; each of the 8 cores takes B/8 = 128 rows
and needs only its own rows of every input (the pairwise interaction is
within-row), so there is no replication and no communication.

Per-core compute layout ("fixed-a" scheme): the 128-row block s_sc = s/tau
lives in SBUF as [128 rows, 1024 cols].  For each a in [0,1024):
  - one ScalarE ACTIVATE computes sig_a[r,b] = sigmoid(-s_sc[r,b] + s_sc[r,a])
    for all 128 rows x 1024 b in one instruction, using the per-partition
    bias operand (bias = column a of s_sc) -- no transpose, no broadcast.
  - TensorE accumulates nf[r,a]*sig_a[r,b] and pf[r,a]*sig_a[r,b] into PSUM
    via matmuls whose stationary operand is diag(nf[:,a]) / diag(pf[:,a])
    (built per-a on VectorE as identity * per-partition-scalar).
"""

import os
import numpy as np

TAU = 0.1
K = 60.0
B = 1024
N_CORES = 8
ROWS = B // N_CORES  # 128

_CACHE = {}

# Fraction of a-steps computed on VectorE via sig = 2^-S/(2^-S + e^(sb-sa-S*ln2))
# (exp blocks precomputed on ScalarE; 2^-S folded into the diag weights).
DVE_EVERY = 6  # a % DVE_EVERY == 0 -> DVE path
SHIFT = 40.0  # power-of-two scale to keep reciprocal inputs in normal range
LN2 = 0.6931471805599453


def _build(n_a):
    from contextlib import ExitStack

    import concourse.bass as bass
    import concourse.bacc as bacc
    import concourse.mybir as mybir
    from concourse import tile

    f32 = mybir.dt.float32
    bf16 = mybir.dt.bfloat16
    AF = mybir.ActivationFunctionType
    OP = mybir.AluOpType

    nc = bacc.Bacc("TRN2", num_devices=N_CORES)

    dins = {
        n: nc.dram_tensor(n, [ROWS, B], f32, kind="ExternalInput")
        for n in ["sv", "sl", "pf", "nf", "wv", "wl"]
    }
    ident_d = nc.dram_tensor("ident", [128, 128], bf16, kind="ExternalInput")
    ident32_d = nc.dram_tensor("ident32", [128, 128], f32, kind="ExternalInput")
    out_d = nc.dram_tensor("out", [ROWS, B], f32, kind="ExternalOutput")
    f32r = mybir.dt.float32r

    with tile.TileContext(nc) as tc:
        with ExitStack() as ctx:
            cpool = ctx.enter_context(tc.tile_pool(name="const", bufs=1))
            sigp = ctx.enter_context(tc.tile_pool(name="sig", bufs=6))
            diagp = ctx.enter_context(tc.tile_pool(name="diag", bufs=6))
            psump = ctx.enter_context(tc.tile_pool(name="acc", bufs=1, space="PSUM"))
            epi = ctx.enter_context(tc.tile_pool(name="epi", bufs=1))

            sb = {}
            for n in ["sv", "sl", "pf", "nf", "wv", "wl"]:
                t = cpool.tile([ROWS, B], f32, tag=n, name=n)
                nc.gpsimd.dma_start(t[:], dins[n][:, :])
                sb[n] = t
            ident = cpool.tile([128, 128], bf16, tag="ident")
            nc.gpsimd.dma_start(ident[:], ident_d[:, :])
            ident32 = cpool.tile([128, 128], f32, tag="ident32")
            nc.gpsimd.dma_start(ident32[:], ident32_d[:, :])

            # Exp blocks for the DVE sigmoid path (all Exp before any Sigmoid
            # so the ACT table set switches exactly once).
            pbias = cpool.tile([128, 1], f32, tag="pbias")
            nc.vector.memset(pbias[:], -SHIFT * LN2)
            qex = {}
            pex = {}
            for sn in ("sv", "sl"):
                q = cpool.tile([ROWS, B], f32, tag="q" + sn, name="q" + sn)
                nc.scalar.activation(q[:], sb[sn][:], AF.Exp)
                qex[sn] = q
                p = cpool.tile([ROWS, B], f32, tag="p" + sn, name="p" + sn)
                nc.scalar.activation(
                    p[:], sb[sn][:], AF.Exp, scale=-1.0, bias=pbias[:]
                )
                pex[sn] = p

            accs = {}
            for n in ["pv", "nv", "pl", "nl"]:
                accs[n] = psump.tile([ROWS, B], f32, tag=n, name=n)

            for a in range(n_a):
                use_dve = (a % DVE_EVERY) == 0
                if use_dve:
                    dN = diagp.tile([128, 128], f32r, tag="diag32", name="dN32")
                    nc.vector.tensor_scalar_mul(dN[:], ident32[:], sb["nf"][:, a : a + 1])
                    dP = diagp.tile([128, 128], f32r, tag="diag32", name="dP32")
                    nc.vector.tensor_scalar_mul(dP[:], ident32[:], sb["pf"][:, a : a + 1])

                else:
                    dN = diagp.tile([128, 128], bf16, tag="diag", name="dN")
                    nc.vector.tensor_scalar_mul(dN[:], ident[:], sb["nf"][:, a : a + 1])
                    dP = diagp.tile([128, 128], bf16, tag="diag", name="dP")
                    nc.vector.tensor_scalar_mul(dP[:], ident[:], sb["pf"][:, a : a + 1])
                for sn, ap_, an_ in (("sv", "pv", "nv"), ("sl", "pl", "nl")):
                    st = sb[sn]
                    if use_dve:
                        t = sigp.tile([ROWS, B], f32, tag="dvet", name="dvet")
                        nc.vector.tensor_scalar(
                            t[:],
                            qex[sn][:],
                            pex[sn][:, a : a + 1],
                            2.0 ** (-SHIFT),
                            OP.mult,
                            OP.add,
                        )
                        inv = sigp.tile([ROWS, B], f32r, tag="dveinv", name="dveinv")
                        with nc.allow_low_precision("f32r for full-rate PE"):
                            nc.vector.reciprocal(inv[:], t[:])
                        sig = inv
                    else:
                        sig = sigp.tile([ROWS, B], bf16, tag="sig", name="sig")
                        nc.scalar.activation(
                            sig[:], st[:], AF.Sigmoid, bias=st[:, a : a + 1], scale=-1.0
                        )
                    for h in (0, 512):
                        nc.tensor.matmul(
                            accs[ap_][:, h : h + 512],
                            dN[:],
                            sig[:, h : h + 512],
                            start=(a == 0),
                            stop=(a == n_a - 1),
                        )
                        nc.tensor.matmul(
                            accs[an_][:, h : h + 512],
                            dP[:],
                            sig[:, h : h + 512],
                            start=(a == 0),
                            stop=(a == n_a - 1),
                        )

            # Epilogue: S_hat = (K+1) * (wv/(K+rank_v) + wl/(K+rank_l))
            terms = []
            for ap_, an_, wn in (("pv", "nv", "wv"), ("pl", "nl", "wl")):
                tp = epi.tile([ROWS, B], f32, tag="tp" + wn)
                nc.vector.scalar_tensor_tensor(
                    tp[:], accs[ap_][:], 1.0, sb["pf"][:], OP.add, OP.mult
                )
                tn = epi.tile([ROWS, B], f32, tag="tn" + wn)
                nc.vector.scalar_tensor_tensor(
                    tn[:], accs[an_][:], 1.0, sb["nf"][:], OP.add, OP.mult
                )
                den = epi.tile([ROWS, B], f32, tag="den" + wn)
                nc.vector.scalar_tensor_tensor(
                    den[:], tp[:], K, tn[:], OP.add, OP.add
                )
                inv = epi.tile([ROWS, B], f32, tag="inv" + wn)
                nc.vector.reciprocal(inv[:], den[:])
                term = epi.tile([ROWS, B], f32, tag="term" + wn)
                nc.vector.tensor_mul(term[:], inv[:], sb[wn][:])
                terms.append(term)
            ssum = epi.tile([ROWS, B], f32, tag="ssum")
            nc.vector.tensor_add(ssum[:], terms[0][:], terms[1][:])
            res = epi.tile([ROWS, B], f32, tag="res")
            nc.vector.tensor_scalar_mul(res[:], ssum[:], K + 1.0)
            nc.sync.dma_start(out_d[:, :], res[:])

    nc.compile()
    return nc


def _get_nc(n_a):
    if n_a not in _CACHE:
        _CACHE[n_a] = _build(n_a)
    return _CACHE[n_a]


def run(s_v, s_l, pos_mask, neg_mask, w_v, w_l, n_a=B, trace=False):
    from concourse.bass_utils import run_bass_kernel_spmd

    in_maps = _prep_in_maps(s_v, s_l, pos_mask, neg_mask, w_v, w_l)
    nc = _get_nc(n_a)
    br = run_bass_kernel_spmd(nc, in_maps, list(range(N_CORES)), trace=trace)
    out = np.concatenate([br.results[i]["out"] for i in range(N_CORES)], axis=0)
    return out, br


def kernel(s_v, s_l, pos_mask, neg_mask, w_v, w_l):
    out, _ = run(s_v, s_l, pos_mask, neg_mask, w_v, w_l)
    return out


def _prep_in_maps(s_v, s_l, pos_mask, neg_mask, w_v, w_l):
    import ml_dtypes

    sv = (np.asarray(s_v, dtype=np.float32) / TAU).astype(np.float32)
    sl = (np.asarray(s_l, dtype=np.float32) / TAU).astype(np.float32)
    pf = np.asarray(pos_mask).astype(np.float32)
    nf = np.asarray(neg_mask).astype(np.float32)
    wv = np.asarray(w_v, dtype=np.float32)
    wl = np.asarray(w_l, dtype=np.float32)
    ident = np.eye(128, dtype=ml_dtypes.bfloat16)
    ident32 = (np.eye(128) * 2.0 ** (-SHIFT)).astype(np.float32)
    in_maps = []
    for i in range(N_CORES):
        r = slice(i * ROWS, (i + 1) * ROWS)
        in_maps.append(
            {"sv": sv[r], "sl": sl[r], "pf": pf[r], "nf": nf[r],
             "wv": wv[r], "wl": wl[r], "ident": ident, "ident32": ident32}
        )
    return in_maps


def make_runner(nc, in_maps):
    """Persistent jitted runner for repeat-timing (mirrors run_bass_via_pjrt)."""
    import jax
    import concourse.mybir as mybir
    from jax.experimental.shard_map import shard_map
    from jax.sharding import Mesh, NamedSharding, PartitionSpec
    from concourse import bass2jax

    bass2jax.install_neuronx_cc_hook()
    n_cores = len(in_maps)
    partition_name = nc.partition_id_tensor.name if nc.partition_id_tensor else None

    in_names, out_names, out_avals, zero_outs = [], [], [], []
    for alloc in nc.m.functions[0].allocations:
        if not isinstance(alloc, mybir.MemoryLocationSet):
            continue
        name = alloc.memorylocations[0].name
        if alloc.kind == "ExternalInput":
            if name != partition_name:
                in_names.append(name)
        elif alloc.kind == "ExternalOutput":
            shape = tuple(alloc.tensor_shape)
            dtype = mybir.dt.np(alloc.dtype)
            out_names.append(name)
            out_avals.append(jax.core.ShapedArray(shape, dtype))
            zero_outs.append(np.zeros(shape, dtype))
    n_params = len(in_names)
    all_in_names = list(in_names) + list(out_names)

    def _body(*args):
        operands = list(args)
        if partition_name is not None:
            operands.append(bass2jax.partition_id_tensor())
        bind_in_names = all_in_names + ([partition_name] if partition_name else [])
        outs = bass2jax._bass_exec_p.bind(
            *operands,
            out_avals=tuple(out_avals),
            in_names=tuple(bind_in_names),
            out_names=tuple(out_names),
            lowering_input_output_aliases=(),
            sim_require_finite=True,
            sim_require_nnan=True,
            nc=nc,
        )
        return tuple(outs)

    devices = jax.devices()[:n_cores]
    mesh = Mesh(np.asarray(devices), ("core",))
    spec = PartitionSpec("core")
    sharded = jax.jit(
        shard_map(
            _body,
            mesh=mesh,
            in_specs=(spec,) * (n_params + len(out_names)),
            out_specs=(spec,) * len(out_names),
            check_rep=False,
        ),
        keep_unused=True,
    )
    sharding = NamedSharding(mesh, spec)
    concat_in = [
        jax.device_put(
            np.concatenate([np.asarray(in_maps[c][n]) for c in range(n_cores)], axis=0),
            sharding,
        )
        for n in in_names
    ]
    concat_zeros = [
        jax.device_put(np.zeros((n_cores * z.shape[0], *z.shape[1:]), z.dtype), sharding)
        for z in zero_outs
    ]

    def exec_once():
        outs = sharded(*concat_in, *concat_zeros)
        jax.block_until_ready(outs)
        return outs

    def fetch(outs):
        res = []
        for c in range(n_cores):
            d = {
                name: np.asarray(outs[i]).reshape(n_cores, *out_avals[i].shape)[c]
                for i, name in enumerate(out_names)
            }
            res.append(d)
        return res

    return exec_once, fetch
